# revision 9
# baseline (speedup 1.0000x reference)
"""Causal self-attention Trainium2 kernel, 8-core SPMD (token-sharded, collective-free).

Model: B=4, T=2048, D=1024, H=16 heads x 64. out = softmax(mask(QK^T/8)) V W_proj^T.

Sharding: 2 cores per batch. Core c handles batch c//2 and the 8 query tiles
(128 tokens each) at real positions t = 2j + (c%2), j=0..7 -- an interleaved
split so the causal work per core is balanced. Each core computes K/V for the
whole batch (modest recompute), attention for its own queries, and the output
projection for its own rows. No collectives; the causal structure difference
between even/odd cores is encoded purely in input data (mask tiles), so the
SPMD program is identical on all cores.

All matmuls run in bf16 (fp32 accumulate). Verified numerics vs the fp32
reference: rel err ~3e-3 (scores are pre-scaled by 1/8 via the Q weights, and
|score| <= ~3.1 so softmax needs no max subtraction).
"""

import os
from contextlib import ExitStack

import numpy as np
import ml_dtypes

import concourse.bass as bass
import concourse.mybir as mybir
import concourse.tile as tile
from concourse import bacc
from concourse.bass_utils import run_bass_kernel_spmd

BF16 = mybir.dt.bfloat16
F32 = mybir.dt.float32
EXP = mybir.ActivationFunctionType.Exp

B, T, D = 4, 2048, 1024
H, DH = 16, 64
NCORES = 8
QT = 8           # q-tiles of 128 per core
KT = 16          # k-tiles of 128 per batch
NPAIR = 8        # head pairs
NEG = -1e9

_cached = {}


def _build_program():
    nc = bacc.Bacc("TRN2", name="causal_attn")

    x_kvT = nc.dram_tensor("x_kvT", [D, T], BF16, kind="ExternalInput")
    x_qT = nc.dram_tensor("x_qT", [D, 1024], BF16, kind="ExternalInput")
    w_qT = nc.dram_tensor("w_qT", [D, D], BF16, kind="ExternalInput")
    w_kT = nc.dram_tensor("w_kT", [D, D], BF16, kind="ExternalInput")
    w_vT = nc.dram_tensor("w_vT", [D, D], BF16, kind="ExternalInput")
    w_pT = nc.dram_tensor("w_pT", [D, D], BF16, kind="ExternalInput")
    maskd = nc.dram_tensor("mask", [128, 256], F32, kind="ExternalInput")
    outd = nc.dram_tensor("out_T", [D, 1024], F32, kind="ExternalOutput")

    with ExitStack() as ctx:
        tc = ctx.enter_context(tile.TileContext(nc))

        # ---- persistent pools (whole kernel) ----
        const = ctx.enter_context(tc.tile_pool(name="const", bufs=1))
        kpool = ctx.enter_context(tc.tile_pool(name="ksb", bufs=1))
        qpool = ctx.enter_context(tc.tile_pool(name="qsb", bufs=1))
        vpool = ctx.enter_context(tc.tile_pool(name="vsb", bufs=1))
        mm_ps = ctx.enter_context(tc.tile_pool(name="mm_ps", bufs=2, space="PSUM"))
        st_ps = ctx.enter_context(tc.tile_pool(name="st_ps", bufs=3, space="PSUM"))
        pv_ps = ctx.enter_context(tc.tile_pool(name="pv_ps", bufs=3, space="PSUM"))

        mask_sb = const.tile([128, 256], F32)
        nc.sync.dma_start(out=mask_sb[:, :], in_=maskd[:, :])

        K_sb = [kpool.tile([128, T], BF16, tag=f"k{p}", name=f"k{p}") for p in range(NPAIR)]
        Q_sb = [qpool.tile([128, 1024], BF16, tag=f"q{p}", name=f"q{p}") for p in range(NPAIR)]
        # V with a ones column per head: [k-tile, 16 heads, 64+1]
        V_sb = [vpool.tile([128, H, DH + 1], BF16, tag=f"v{m}", name=f"v{m}") for m in range(KT)]

        with ExitStack() as s1:
            xkvp = s1.enter_context(tc.tile_pool(name="xkv", bufs=1))
            wvp = s1.enter_context(tc.tile_pool(name="wv", bufs=1))
            xkv = [xkvp.tile([128, T], BF16, tag=f"xkv{d}", name=f"xkv{d}") for d in range(8)]
            wv = [wvp.tile([128, D], BF16, tag=f"wv{d}", name=f"wv{d}") for d in range(8)]
            for d in range(8):
                nc.sync.dma_start(out=xkv[d][:, :], in_=x_kvT[128 * d:128 * d + 128, :])
                nc.sync.dma_start(out=wv[d][:, :], in_=w_vT[128 * d:128 * d + 128, :])

            with ExitStack() as s2:
                xqp = s2.enter_context(tc.tile_pool(name="xq", bufs=1))
                wqp = s2.enter_context(tc.tile_pool(name="wq", bufs=1))
                wkp = s2.enter_context(tc.tile_pool(name="wk", bufs=1))
                xq = [xqp.tile([128, 1024], BF16, tag=f"xq{d}", name=f"xq{d}") for d in range(8)]
                wq = [wqp.tile([128, D], BF16, tag=f"wq{d}", name=f"wq{d}") for d in range(8)]
                wk = [wkp.tile([128, D], BF16, tag=f"wk{d}", name=f"wk{d}") for d in range(8)]
                for d in range(8):
                    nc.sync.dma_start(out=xq[d][:, :], in_=x_qT[128 * d:128 * d + 128, :])
                    nc.sync.dma_start(out=wq[d][:, :], in_=w_qT[128 * d:128 * d + 128, :])
                    nc.sync.dma_start(out=wk[d][:, :], in_=w_kT[128 * d:128 * d + 128, :])

                # K^T [pair 128, T] and Q^T [pair 128, 1024] (form 2: W stationary)
                for p in range(NPAIR):
                    for n in range(4):
                        ps = mm_ps.tile([128, 512], F32)
                        for d in range(8):
                            nc.tensor.matmul(
                                ps[:, :],
                                lhsT=wk[d][:, 128 * p:128 * p + 128],
                                rhs=xkv[d][:, 512 * n:512 * n + 512],
                                start=(d == 0), stop=(d == 7),
                            )
                        nc.vector.tensor_copy(K_sb[p][:, 512 * n:512 * n + 512], ps[:, :])
                    for n in range(2):
                        ps = mm_ps.tile([128, 512], F32)
                        for d in range(8):
                            nc.tensor.matmul(
                                ps[:, :],
                                lhsT=wq[d][:, 128 * p:128 * p + 128],
                                rhs=xq[d][:, 512 * n:512 * n + 512],
                                start=(d == 0), stop=(d == 7),
                            )
                        nc.vector.tensor_copy(Q_sb[p][:, 512 * n:512 * n + 512], ps[:, :])

            # V natural layout (form 1: x stationary), strided into V_sb
            if True:
                for m in range(KT):
                    for n in range(2):
                        ps = mm_ps.tile([128, 512], F32)
                        for d in range(8):
                            nc.tensor.matmul(
                                ps[:, :],
                                lhsT=xkv[d][:, 128 * m:128 * m + 128],
                                rhs=wv[d][:, 512 * n:512 * n + 512],
                                start=(d == 0), stop=(d == 7),
                            )
                        nc.scalar.copy(
                            V_sb[m][:, 8 * n:8 * n + 8, 0:DH],
                            ps[:, :].rearrange("p (h e) -> p h e", h=8),
                        )
                    nc.vector.memset(V_sb[m][:, :, DH:DH + 1], 1.0)

        # attention-phase pools reuse the QKV-stage SBUF; one barrier keeps the
        # WAR fan-in to a single sync point (walrus caps per-DMA wait count)
        tc.strict_bb_all_engine_barrier()
        opool = ctx.enter_context(tc.tile_pool(name="osb", bufs=1))
        O_sb = [opool.tile([128, 1024], BF16, tag=f"o{p}", name=f"o{p}") for p in range(NPAIR)]
        ppool = ctx.enter_context(tc.tile_pool(name="pex", bufs=16))
        rpool = ctx.enter_context(tc.tile_pool(name="recip", bufs=3))
        bpool = ctx.enter_context(tc.tile_pool(name="bcast", bufs=3))
        tpool = ctx.enter_context(tc.tile_pool(name="otmp", bufs=3))
        drp = ctx.enter_context(tc.tile_pool(name="rscratch", bufs=3, space="DRAM"))
        outsb = ctx.enter_context(tc.tile_pool(name="outsb", bufs=3))
        # w_pT prefetch overlaps attention
        wpp = ctx.enter_context(tc.tile_pool(name="wp", bufs=1))
        wp = [wpp.tile([128, D], BF16, tag=f"wp{d}", name=f"wp{d}") for d in range(8)]
        for d in range(8):
            nc.sync.dma_start(out=wp[d][:, :], in_=w_pT[128 * d:128 * d + 128, :])

        # ---- attention ----
        def qk(p, h_off, ki, q0, qw, st_out):
            nc.tensor.matmul(
                st_out,
                lhsT=K_sb[p][h_off:h_off + 64, 128 * ki:128 * ki + 128],
                rhs=Q_sb[p][h_off:h_off + 64, q0:q0 + qw],
                start=True, stop=True,
            )

        for p in range(NPAIR):
            for J in range(2):
                q0 = 512 * J
                nbulk = 8 * J
                for hi, h_off in ((0, 0), (1, 64)):
                    h = 2 * p + hi
                    pv = pv_ps.tile([65, 512], F32)
                    pieces = []  # (j_region or None, list of (ki, P_ap))

                    # bulk k-tiles: full 512-wide, no mask
                    for ki in range(nbulk):
                        st = st_ps.tile([128, 512], F32)
                        qk(p, h_off, ki, q0, 512, st[:, :])
                        pb = ppool.tile([128, 512], BF16)
                        nc.scalar.activation(pb[:, :], st[:, :], EXP)
                        # psum start/stop are bank-granular: start only on the
                        # very first matmul into pv, stop only on the last
                        nc.tensor.matmul(
                            pv[:, :],
                            lhsT=V_sb[ki][:, h, :],
                            rhs=pb[:, :],
                            start=(ki == 0), stop=False,
                        )

                    # extras: per q-tile j, k-tiles [nbulk, 2j+2), last two masked
                    for j in range(4 * J, 4 * J + 4):
                        kis = list(range(nbulk, 2 * j + 2))
                        qc = 128 * (j - 4 * J)  # column offset inside this 512 chunk
                        # group extras into [128, 512] st tiles (4 slots each)
                        for g0 in range(0, len(kis), 4):
                            grp = kis[g0:g0 + 4]
                            st = st_ps.tile([128, 512], F32)
                            for s, ki in enumerate(grp):
                                qk(p, h_off, ki, q0 + qc, 128, st[:, 128 * s:128 * s + 128])
                                m = ki - 2 * j  # -> 0 or 1 for the last two
                                if m >= 0:
                                    nc.vector.tensor_add(
                                        st[:, 128 * s:128 * s + 128],
                                        st[:, 128 * s:128 * s + 128],
                                        mask_sb[:, 128 * m:128 * m + 128],
                                    )
                            px = ppool.tile([128, 512], BF16)
                            nw = 128 * len(grp)
                            nc.scalar.activation(px[:, :nw], st[:, :nw], EXP)
                            for s, ki in enumerate(grp):
                                nc.tensor.matmul(
                                    pv[:, qc:qc + 128],
                                    lhsT=V_sb[ki][:, h, :],
                                    rhs=px[:, 128 * s:128 * s + 128],
                                    start=(nbulk == 0 and j == 4 * J and g0 == 0 and s == 0),
                                    stop=(j == 4 * J + 3 and ki == kis[-1]),
                                )

                    # normalize: row 64 of pv holds the softmax denominators
                    rt = rpool.tile([65, 512], F32)
                    nc.vector.reciprocal(rt[64:65, :], pv[64:65, :])
                    rd = drp.tile([512], F32)
                    nc.sync.dma_start(out=rd[:], in_=rt[64:65, :])
                    bc = bpool.tile([64, 512], F32)
                    nc.sync.dma_start(
                        out=bc[:, :],
                        in_=bass.AP(tensor=rd.tensor, offset=rd.offset,
                                    ap=[[0, 64]] + list(rd.ap)),
                    )
                    if hi == 0:
                        nc.vector.tensor_mul(O_sb[p][0:64, q0:q0 + 512], pv[0:64, :], bc[:, :])
                    else:
                        ot = tpool.tile([64, 512], BF16)
                        nc.vector.tensor_mul(ot[:, :], pv[0:64, :], bc[:, :])
                        nc.sync.dma_start(out=O_sb[p][64:128, q0:q0 + 512], in_=ot[:, :])

        # ---- output projection: out_T[o, q] = sum_d w_pT[d, o]^T O[d, q] ----
        for m in range(8):
            for n in range(2):
                ps = mm_ps.tile([128, 512], F32)
                for p in range(NPAIR):
                    nc.tensor.matmul(
                        ps[:, :],
                        lhsT=wp[p][:, 128 * m:128 * m + 128],
                        rhs=O_sb[p][:, 512 * n:512 * n + 512],
                        start=(p == 0), stop=(p == 7),
                    )
                ob = outsb.tile([128, 512], F32)
                nc.scalar.copy(ob[:, :], ps[:, :])
                nc.sync.dma_start(
                    out=outd[128 * m:128 * m + 128, 512 * n:512 * n + 512],
                    in_=ob[:, :],
                )

    nc.finalize()
    return nc


def _host_inputs(x, W_qkv, W_proj):
    bf = ml_dtypes.bfloat16
    wq = np.ascontiguousarray((W_qkv[0:D] / 8.0).T.astype(bf))
    wk = np.ascontiguousarray(W_qkv[D:2 * D].T.astype(bf))
    wv = np.ascontiguousarray(W_qkv[2 * D:3 * D].T.astype(bf))
    wp = np.ascontiguousarray(W_proj.T.astype(bf))

    kk, qq = np.meshgrid(np.arange(128), np.arange(128), indexing="ij")
    stair = np.where(kk <= qq, 0.0, NEG).astype(np.float32)
    masks = {
        0: np.concatenate([stair, np.full((128, 128), NEG, np.float32)], axis=1),
        1: np.concatenate([np.zeros((128, 128), np.float32), stair], axis=1),
    }

    in_maps = []
    for c in range(NCORES):
        b, fold = c // 2, c % 2
        xT = np.ascontiguousarray(x[b].T.astype(bf))  # [D, T]
        qidx = np.concatenate(
            [np.arange(128 * (2 * j + fold), 128 * (2 * j + fold) + 128) for j in range(QT)]
        )
        in_maps.append({
            "x_kvT": xT,
            "x_qT": np.ascontiguousarray(xT[:, qidx]),
            "w_qT": wq, "w_kT": wk, "w_vT": wv, "w_pT": wp,
            "mask": np.ascontiguousarray(masks[fold]),
        })
    return in_maps


def _run(inputs, trace=False, trace_cores=None):
    if "nc" not in _cached:
        _cached["nc"] = _build_program()
    nc = _cached["nc"]
    in_maps = _host_inputs(inputs["x"], inputs["W_qkv"], inputs["W_proj"])
    res = run_bass_kernel_spmd(
        nc, in_maps, core_ids=list(range(NCORES)),
        trace=trace, trace_cores=trace_cores,
    )
    out = np.zeros((B, T, D), np.float32)
    for c in range(NCORES):
        b, fold = c // 2, c % 2
        oT = res.results[c]["out_T"]  # [D, 1024]
        for j in range(QT):
            t0 = 128 * (2 * j + fold)
            out[b, t0:t0 + 128, :] = oT[:, 128 * j:128 * j + 128].T
    return out, res


def kernel(**inputs) -> np.ndarray:
    out, _ = _run(inputs, trace=os.environ.get("KERNEL_TRACE", "") == "1")
    return out


# revision 15
# speedup vs baseline: 1.1917x; 1.1917x over previous
"""Causal self-attention Trainium2 kernel, 8-core SPMD (token-sharded, collective-free).

Model: B=4, T=2048, D=1024, H=16 heads x 64. out = softmax(mask(QK^T/8)) V W_proj^T.

Sharding: 2 cores per batch. Core c handles batch c//2 and the 8 query tiles
(128 tokens each) at real positions t = 2j + (c%2), j=0..7 -- an interleaved
split so the causal work per core is balanced. Each core computes K/V for the
whole batch (modest recompute), attention for its own queries, and the output
projection for its own rows. No collectives; the causal structure difference
between even/odd cores is encoded purely in input data (mask tiles), so the
SPMD program is identical on all cores.

All matmuls run in bf16 (fp32 accumulate). Verified numerics vs the fp32
reference: rel err ~3e-3 (scores are pre-scaled by 1/8 via the Q weights, and
|score| <= ~3.1 so softmax needs no max subtraction).
"""

import os
from contextlib import ExitStack

import numpy as np
import ml_dtypes

import concourse.bass as bass
import concourse.mybir as mybir
import concourse.tile as tile
from concourse import bacc
from concourse.bass_utils import run_bass_kernel_spmd

BF16 = mybir.dt.bfloat16
F32 = mybir.dt.float32
EXP = mybir.ActivationFunctionType.Exp

B, T, D = 4, 2048, 1024
H, DH = 16, 64
NCORES = 8
QT = 8           # q-tiles of 128 per core
KT = 16          # k-tiles of 128 per batch
NPAIR = 8        # head pairs
NEG = -1e9

_cached = {}

if os.environ.get("BASS_LDW_OPT", "") == "1":
    # A/B experiment: let walrus keep/overlap LDWEIGHTS (default path passes
    # --enable-ldw-opt=false)
    from concourse import bass_utils as _bu
    _orig_run_command = _bu.run_command
    def _patched_run_command(argv, **kwargs):
        argv = [a.replace("--enable-ldw-opt=false", "--enable-ldw-opt=true")
                if isinstance(a, str) else a for a in argv]
        return _orig_run_command(argv, **kwargs)
    _bu.run_command = _patched_run_command


def _build_program():
    nc = bacc.Bacc("TRN2", name="causal_attn")

    x_kvT = nc.dram_tensor("x_kvT", [D, T], BF16, kind="ExternalInput")
    x_qT = nc.dram_tensor("x_qT", [D, 1024], BF16, kind="ExternalInput")
    w_qT = nc.dram_tensor("w_qT", [D, D], BF16, kind="ExternalInput")
    w_kT = nc.dram_tensor("w_kT", [D, D], BF16, kind="ExternalInput")
    w_vT = nc.dram_tensor("w_vT", [D, D], BF16, kind="ExternalInput")
    w_pT = nc.dram_tensor("w_pT", [D, D], BF16, kind="ExternalInput")
    maskd = nc.dram_tensor("mask", [128, 256], BF16, kind="ExternalInput")
    outd = nc.dram_tensor("out_T", [D, 1024], F32, kind="ExternalOutput")

    with ExitStack() as ctx:
        tc = ctx.enter_context(tile.TileContext(nc))

        # ---- persistent pools (whole kernel) ----
        const = ctx.enter_context(tc.tile_pool(name="const", bufs=1))
        kpool = ctx.enter_context(tc.tile_pool(name="ksb", bufs=1))
        qpool = ctx.enter_context(tc.tile_pool(name="qsb", bufs=1))
        vpool = ctx.enter_context(tc.tile_pool(name="vsb", bufs=1))
        mm_ps = ctx.enter_context(tc.tile_pool(name="mm_ps", bufs=5, space="PSUM"))
        st_ps = ctx.enter_context(tc.tile_pool(name="st_ps", bufs=3, space="PSUM"))
        pv_ps = mm_ps

        mask_sb = const.tile([128, 256], BF16)
        nc.sync.dma_start(out=mask_sb[:, :], in_=maskd[:, :])

        K_sb = [kpool.tile([128, T], BF16, tag=f"k{p}", name=f"k{p}") for p in range(NPAIR)]
        Q_sb = [qpool.tile([128, 1024], BF16, tag=f"q{p}", name=f"q{p}") for p in range(NPAIR)]
        # V with a ones column per head: [k-tile, 16 heads, 64+1]
        V_sb = [vpool.tile([128, H, DH + 1], BF16, tag=f"v{m}", name=f"v{m}") for m in range(KT)]

        with ExitStack() as s1:
            xkvp = s1.enter_context(tc.tile_pool(name="xkv", bufs=1))
            wvp = s1.enter_context(tc.tile_pool(name="wv", bufs=1))
            xkv = [xkvp.tile([128, T], BF16, tag=f"xkv{d}", name=f"xkv{d}") for d in range(8)]
            wv = [wvp.tile([128, D], BF16, tag=f"wv{d}", name=f"wv{d}") for d in range(8)]
            for d in range(8):
                nc.sync.dma_start(out=xkv[d][:, :], in_=x_kvT[128 * d:128 * d + 128, :])
                nc.sync.dma_start(out=wv[d][:, :], in_=w_vT[128 * d:128 * d + 128, :])

            with ExitStack() as s2:
                xqp = s2.enter_context(tc.tile_pool(name="xq", bufs=1))
                wqp = s2.enter_context(tc.tile_pool(name="wq", bufs=1))
                wkp = s2.enter_context(tc.tile_pool(name="wk", bufs=1))
                xq = [xqp.tile([128, 1024], BF16, tag=f"xq{d}", name=f"xq{d}") for d in range(8)]
                wq = [wqp.tile([128, D], BF16, tag=f"wq{d}", name=f"wq{d}") for d in range(8)]
                wk = [wkp.tile([128, D], BF16, tag=f"wk{d}", name=f"wk{d}") for d in range(8)]
                for d in range(8):
                    nc.sync.dma_start(out=xq[d][:, :], in_=x_qT[128 * d:128 * d + 128, :])
                    nc.sync.dma_start(out=wq[d][:, :], in_=w_qT[128 * d:128 * d + 128, :])
                    nc.sync.dma_start(out=wk[d][:, :], in_=w_kT[128 * d:128 * d + 128, :])

                # K^T [pair 128, T] and Q^T [pair 128, 1024] (form 2: W stationary)
                for p in range(NPAIR):
                    for n in range(4):
                        ps = mm_ps.tile([128, 512], F32, tag="ps", name="ps")
                        for d in range(8):
                            nc.tensor.matmul(
                                ps[:, :],
                                lhsT=wk[d][:, 128 * p:128 * p + 128],
                                rhs=xkv[d][:, 512 * n:512 * n + 512],
                                start=(d == 0), stop=(d == 7),
                            )
                        nc.vector.tensor_copy(K_sb[p][:, 512 * n:512 * n + 512], ps[:, :])
                    for n in range(2):
                        ps = mm_ps.tile([128, 512], F32, tag="ps", name="ps")
                        for d in range(8):
                            nc.tensor.matmul(
                                ps[:, :],
                                lhsT=wq[d][:, 128 * p:128 * p + 128],
                                rhs=xq[d][:, 512 * n:512 * n + 512],
                                start=(d == 0), stop=(d == 7),
                            )
                        nc.vector.tensor_copy(Q_sb[p][:, 512 * n:512 * n + 512], ps[:, :])

            # V natural layout (form 1: x stationary), strided into V_sb
            if True:
                for m in range(KT):
                    for n in range(2):
                        ps = mm_ps.tile([128, 512], F32, tag="ps", name="ps")
                        for d in range(8):
                            nc.tensor.matmul(
                                ps[:, :],
                                lhsT=xkv[d][:, 128 * m:128 * m + 128],
                                rhs=wv[d][:, 512 * n:512 * n + 512],
                                start=(d == 0), stop=(d == 7),
                            )
                        nc.scalar.copy(
                            V_sb[m][:, 8 * n:8 * n + 8, 0:DH],
                            ps[:, :].rearrange("p (h e) -> p h e", h=8),
                        )
                    nc.vector.memset(V_sb[m][:, :, DH:DH + 1], 1.0)

        # attention-phase pools reuse the QKV-stage SBUF; one barrier keeps the
        # WAR fan-in to a single sync point (walrus caps per-DMA wait count)
        tc.strict_bb_all_engine_barrier()
        opool = ctx.enter_context(tc.tile_pool(name="osb", bufs=1))
        O_sb = [opool.tile([128, 1024], BF16, tag=f"o{p}", name=f"o{p}") for p in range(NPAIR)]
        ppool = ctx.enter_context(tc.tile_pool(name="pex", bufs=16))
        rpool = ctx.enter_context(tc.tile_pool(name="recip", bufs=3))
        bpool = ctx.enter_context(tc.tile_pool(name="bcast", bufs=3))
        tpool = ctx.enter_context(tc.tile_pool(name="otmp", bufs=3))
        drp = ctx.enter_context(tc.tile_pool(name="rscratch", bufs=3, space="DRAM"))
        outsb = ctx.enter_context(tc.tile_pool(name="outsb", bufs=3))
        # w_pT prefetch overlaps attention
        wpp = ctx.enter_context(tc.tile_pool(name="wp", bufs=1))
        wp = [wpp.tile([128, D], BF16, tag=f"wp{d}", name=f"wp{d}") for d in range(8)]
        for d in range(8):
            nc.sync.dma_start(out=wp[d][:, :], in_=w_pT[128 * d:128 * d + 128, :])

        # ---- attention ----
        def qk(p, h_off, ki, q0, qw, st_out):
            nc.tensor.matmul(
                st_out,
                lhsT=K_sb[p][h_off:h_off + 64, 128 * ki:128 * ki + 128],
                rhs=Q_sb[p][h_off:h_off + 64, q0:q0 + qw],
                start=True, stop=True,
            )

        for p in range(NPAIR):
            for J in range(2):
                q0 = 512 * J
                nbulk = 8 * J
                for hi, h_off in ((0, 0), (1, 64)):
                    h = 2 * p + hi
                    pv = pv_ps.tile([65, 512], F32, tag="ps", name="pv")
                    pieces = []  # (j_region or None, list of (ki, P_ap))

                    # bulk k-tiles: full 512-wide, no mask
                    for ki in range(nbulk):
                        st = st_ps.tile([128, 512], F32)
                        qk(p, h_off, ki, q0, 512, st[:, :])
                        pb = ppool.tile([128, 512], BF16)
                        nc.scalar.activation(pb[:, :], st[:, :], EXP)
                        # psum start/stop are bank-granular: start only on the
                        # very first matmul into pv, stop only on the last
                        nc.tensor.matmul(
                            pv[:, :],
                            lhsT=V_sb[ki][:, h, :],
                            rhs=pb[:, :],
                            start=(ki == 0), stop=False,
                        )

                    # extras: per q-tile j, k-tiles [nbulk, 2j+2), last two masked
                    for j in range(4 * J, 4 * J + 4):
                        kis = list(range(nbulk, 2 * j + 2))
                        qc = 128 * (j - 4 * J)  # column offset inside this 512 chunk
                        # group extras into [128, 512] st tiles (4 slots each)
                        for g0 in range(0, len(kis), 4):
                            grp = kis[g0:g0 + 4]
                            st = st_ps.tile([128, 512], F32)
                            for s, ki in enumerate(grp):
                                qk(p, h_off, ki, q0 + qc, 128, st[:, 128 * s:128 * s + 128])
                            px = ppool.tile([128, 512], BF16)
                            nw = 128 * len(grp)
                            nc.scalar.activation(px[:, :nw], st[:, :nw], EXP)
                            for s, ki in enumerate(grp):
                                m = ki - 2 * j  # -> 0 or 1 for the last two
                                if m >= 0:
                                    # multiplicative 0/1 causal mask, bf16 in SBUF
                                    nc.vector.tensor_mul(
                                        px[:, 128 * s:128 * s + 128],
                                        px[:, 128 * s:128 * s + 128],
                                        mask_sb[:, 128 * m:128 * m + 128],
                                    )
                            for s, ki in enumerate(grp):
                                nc.tensor.matmul(
                                    pv[:, qc:qc + 128],
                                    lhsT=V_sb[ki][:, h, :],
                                    rhs=px[:, 128 * s:128 * s + 128],
                                    start=(nbulk == 0 and j == 4 * J and g0 == 0 and s == 0),
                                    stop=(j == 4 * J + 3 and ki == kis[-1]),
                                )

                    # normalize: row 64 of pv holds the softmax denominators;
                    # bounce it through DRAM to broadcast across partitions,
                    # then elementwise divide (no reciprocal pass needed)
                    # sums row -> DRAM -> respread [128,4] so the plain
                    # reciprocal runs 128-wide instead of on one lane, then
                    # -> DRAM -> partition-broadcast [64,512]
                    rt = rpool.tile([65, 512], F32)
                    nc.scalar.copy(rt[64:65, :], pv[64:65, :])
                    rd = drp.tile([512], F32, name="rd")
                    nc.sync.dma_start(out=rd[:], in_=rt[64:65, :])
                    rs = rpool.tile([128, 4], F32, name="rs")
                    nc.sync.dma_start(out=rs[:, :], in_=rd.rearrange("(p f) -> p f", p=128))
                    rs2 = rpool.tile([128, 4], F32, name="rs2")
                    nc.vector.reciprocal(rs2[:, :], rs[:, :])
                    rd2 = drp.tile([512], F32, name="rd2")
                    nc.sync.dma_start(out=rd2.rearrange("(p f) -> p f", p=128), in_=rs2[:, :])
                    bc = bpool.tile([64, 512], F32)
                    nc.sync.dma_start(
                        out=bc[:, :],
                        in_=bass.AP(tensor=rd2.tensor, offset=rd2.offset,
                                    ap=[[0, 64]] + list(rd2.ap)),
                    )
                    if hi == 0:
                        nc.vector.tensor_mul(O_sb[p][0:64, q0:q0 + 512], pv[0:64, :], bc[:, :])
                    else:
                        ot = tpool.tile([64, 512], BF16)
                        nc.vector.tensor_mul(ot[:, :], pv[0:64, :], bc[:, :])
                        nc.sync.dma_start(out=O_sb[p][64:128, q0:q0 + 512], in_=ot[:, :])

        # ---- output projection: out_T[o, q] = sum_d w_pT[d, o]^T O[d, q] ----
        for m in range(8):
            for n in range(2):
                ps = mm_ps.tile([128, 512], F32, tag="ps", name="ps")
                for p in range(NPAIR):
                    nc.tensor.matmul(
                        ps[:, :],
                        lhsT=wp[p][:, 128 * m:128 * m + 128],
                        rhs=O_sb[p][:, 512 * n:512 * n + 512],
                        start=(p == 0), stop=(p == 7),
                    )
                ob = outsb.tile([128, 512], F32)
                nc.scalar.copy(ob[:, :], ps[:, :])
                nc.sync.dma_start(
                    out=outd[128 * m:128 * m + 128, 512 * n:512 * n + 512],
                    in_=ob[:, :],
                )

    nc.finalize()
    return nc


def _host_inputs(x, W_qkv, W_proj):
    bf = ml_dtypes.bfloat16
    wq = np.ascontiguousarray((W_qkv[0:D] / 8.0).T.astype(bf))
    wk = np.ascontiguousarray(W_qkv[D:2 * D].T.astype(bf))
    wv = np.ascontiguousarray(W_qkv[2 * D:3 * D].T.astype(bf))
    wp = np.ascontiguousarray(W_proj.T.astype(bf))

    kk, qq = np.meshgrid(np.arange(128), np.arange(128), indexing="ij")
    stair = (kk <= qq).astype(np.float32)
    masks = {
        0: np.concatenate([stair, np.zeros((128, 128), np.float32)], axis=1).astype(bf),
        1: np.concatenate([np.ones((128, 128), np.float32), stair], axis=1).astype(bf),
    }

    in_maps = []
    for c in range(NCORES):
        b, fold = c // 2, c % 2
        xT = np.ascontiguousarray(x[b].T.astype(bf))  # [D, T]
        qidx = np.concatenate(
            [np.arange(128 * (2 * j + fold), 128 * (2 * j + fold) + 128) for j in range(QT)]
        )
        in_maps.append({
            "x_kvT": xT,
            "x_qT": np.ascontiguousarray(xT[:, qidx]),
            "w_qT": wq, "w_kT": wk, "w_vT": wv, "w_pT": wp,
            "mask": np.ascontiguousarray(masks[fold]),
        })
    return in_maps


def _run(inputs, trace=False, trace_cores=None):
    if "nc" not in _cached:
        _cached["nc"] = _build_program()
    nc = _cached["nc"]
    in_maps = _host_inputs(inputs["x"], inputs["W_qkv"], inputs["W_proj"])
    res = run_bass_kernel_spmd(
        nc, in_maps, core_ids=list(range(NCORES)),
        trace=trace, trace_cores=trace_cores,
    )
    out = np.zeros((B, T, D), np.float32)
    for c in range(NCORES):
        b, fold = c // 2, c % 2
        oT = res.results[c]["out_T"]  # [D, 1024]
        for j in range(QT):
            t0 = 128 * (2 * j + fold)
            out[b, t0:t0 + 128, :] = oT[:, 128 * j:128 * j + 128].T
    return out, res


def kernel(**inputs) -> np.ndarray:
    out, _ = _run(inputs, trace=os.environ.get("KERNEL_TRACE", "") == "1")
    return out
